# revision 34
# baseline (speedup 1.0000x reference)
"""Trainium2 Bass kernel for a graph-GRU (GRNN) forecast model.

Math (per batch b, node m, hidden h; N=2048, H=64, F=2, T=12, P=6):
  ht[b,m,:] = sum_n adj[n,m] * h[b,:,n]           (graph diffusion + transpose)
  r = sig(ht@Ur^T + inp@Wr^T + br); z = sig(...); nw = tanh(r*(ht@Un^T+bn1) + inp@Wn^T + bn2)
  h' = (1-z)*nw + z*ht
Encoder: inp = x_t (T steps). Decoder: out = fc(h); inp = [out, 0] (P steps).

Strategy: data-parallel over batch, 8 cores x 8 batches. Per core the state
lives in SBUF in two layouts:
  hT8 [n=2048(part,16 tiles), (b,h)=512(free)] fp8  - lhsT for the diffusion matmul
  hS  [(bl,h)=128(part), pt=4, m=2048]         bf16 - standard layout
with b = 2*pt + bl (batch-pair pt on separate partition tiles).

Key tricks vs a plain bf16 implementation:
 - adj col m = s_m*(mask + diag(d_m/s_m)) with mask in {0,1}: choosing fp8-exact
   pairs (u,v) with v/u == d_m/s_m (e.g. 10/3 -> u=1.125, v=3.75) makes the
   whole fp8 matrix EXACT, so the diffusion runs as fp8 DoubleRow matmuls
   (K=256/inst); the per-column correction s_m/u_m rides the one mandatory
   PSUM->SBUF multiply on DVE.
 - the per-step state transpose uses the DMA xbar (dma_start_transpose) plus a
   ScalarE Copy-cast to fp8, freeing TensorE (~300us of transpose matmuls).
 - gate biases ride ones-rows inside the input-projection matmuls; the three
   tiny input projections (K=17 enc / K=9 dec) are row-tiled onto disjoint
   32-row PE strips so they run concurrently (~1 matmul time instead of 3).
 - decoder input term uses the fc output (computed once per m-chunk, M=96 with
   3 replicated strips) as a K=9 rank-1 matmul instead of full K=128 matmuls.
"""

import numpy as np
import ml_dtypes

B, T, F, N, H, P = 64, 12, 2, 2048, 64, 6
NCORES = 8
BC = B // NCORES          # batches per core = 8
NPT = BC // 2             # batch-pair tiles = 4
KT = N // 128             # contraction tiles = 16
NMC = N // 512            # m chunks = 4

# If True, fold diag(adj)/s into the fp8 mask (one fp8 rounding per column,
# ~2% on the 17% diagonal mass); saves two elementwise ops per chunk.
DIAG_IN_MASK = True

_BF16 = ml_dtypes.bfloat16
_F8 = ml_dtypes.float8_e4m3

_compiled = None


def _build_bass():
    import concourse.bass as bass
    import concourse.mybir as mybir
    from concourse import bacc
    import concourse.tile as tile

    bf16 = mybir.dt.bfloat16
    f32 = mybir.dt.float32
    fp8 = mybir.dt.float8e4
    AF = mybir.ActivationFunctionType
    ALU = mybir.AluOpType
    DR = mybir.MatmulPerfMode.DoubleRow

    nc = bacc.Bacc(None, target_bir_lowering=False)

    x_d = nc.dram_tensor("xaug", [T, 96, N], bf16, kind="ExternalInput")
    h0s_d = nc.dram_tensor("h0s", [128, NPT, N], bf16, kind="ExternalInput")
    h0t_d = nc.dram_tensor("h0t", [128, KT, 512], fp8, kind="ExternalInput")
    mask_d = nc.dram_tensor("mask8", [128, KT, N], fp8, kind="ExternalInput")
    ublk_d = nc.dram_tensor("ublk", [128, 3, 128], bf16, kind="ExternalInput")
    wx3_d = nc.dram_tensor("wx3", [96, NPT * 3, 128], bf16, kind="ExternalInput")
    wdec_d = nc.dram_tensor("wdec", [96, NPT * 3, 128], bf16, kind="ExternalInput")
    fcb_d = nc.dram_tensor("fcblk", [128, NPT, 96], bf16, kind="ExternalInput")
    ds_d = nc.dram_tensor("dsB", [128, N], f32, kind="ExternalInput")
    s_d = nc.dram_tensor("sB", [128, N], f32, kind="ExternalInput")
    bias_d = nc.dram_tensor("biases", [128, 4], f32, kind="ExternalInput")
    out_d = nc.dram_tensor("out", [BC, P, N], f32, kind="ExternalOutput")

    with tile.TileContext(nc) as tc:
        with (
            tc.tile_pool(name="const", bufs=1) as cp,
            tc.tile_pool(name="state", bufs=1) as sp,
            tc.tile_pool(name="work", bufs=3) as wp,
            tc.tile_pool(name="o8p", bufs=4) as op,
            tc.tile_pool(name="xp", bufs=2) as xp,
            tc.tile_pool(name="tb", bufs=3) as tb,
            tc.tile_pool(name="dps", bufs=3, space="PSUM") as dpool,
            tc.tile_pool(name="rz", bufs=2, space="PSUM") as rzpool,
            tc.tile_pool(name="nunw", bufs=1, space="PSUM") as nupool,
        ):
            # load order matters: the step-0 diffusion needs h0t + mask first
            hT = [sp.tile([128, KT, 512], fp8, name=f"hT{i}") for i in range(2)]
            hS = [sp.tile([128, NPT, N], bf16, name=f"hS{i}") for i in range(2)]
            nc.sync.dma_start(hT[0][:], h0t_d[:])
            mask_sb = cp.tile([128, KT, N], fp8)
            for kt in range(KT):
                eng = nc.sync if kt % 2 == 0 else nc.scalar
                eng.dma_start(mask_sb[:, kt, :], mask_d[:, kt, :])
            ublk = cp.tile([128, 3, 128], bf16)
            nc.sync.dma_start(ublk[:], ublk_d[:])
            wx3 = cp.tile([96, NPT * 3, 128], bf16)
            nc.sync.dma_start(wx3[:], wx3_d[:])
            sB = cp.tile([128, N], f32)
            nc.sync.dma_start(sB[:], s_d[:])
            biases = cp.tile([128, 4], f32)
            nc.sync.dma_start(biases[:], bias_d[:])
            wdec = cp.tile([96, NPT * 3, 128], bf16)
            nc.sync.dma_start(wdec[:], wdec_d[:])
            fcblk = cp.tile([128, NPT, 96], bf16)
            nc.sync.dma_start(fcblk[:], fcb_d[:])
            dsB = cp.tile([128, N], f32)
            nc.sync.dma_start(dsB[:], ds_d[:])
            nc.sync.dma_start(hS[0][:], h0s_d[:])

            for s in range(T + P):
                cur, nxt = s % 2, (s + 1) % 2
                dec = s >= T
                hTc, hTn = hT[cur], hT[nxt]
                hSp, hSn = hS[cur], hS[nxt]
                if not dec:
                    x_sb = xp.tile([96, N], bf16, tag="xsb")
                    nc.sync.dma_start(x_sb[:], x_d[s, :, :])
                else:
                    p_idx = s - T
                def emit_fc(mc):
                    # fc out-projection: M=96 (3 replicated 8-batch strips
                    # + ones-rows via bias), accumulated over pt.
                    # Borrows the rps bank (cleared by the next start=True).
                    ms = slice(mc * 512, (mc + 1) * 512)
                    fct = rzpool.tile([128, 512], f32, tag="rps", name="fct")
                    fcps = fct[0:96, :]
                    for pt in range(NPT):
                        nc.tensor.matmul(
                            fcps[:], fcblk[:, pt, :], hSp[:, pt, ms],
                            start=(pt == 0), stop=(pt == NPT - 1),
                        )
                    o8 = op.tile([96, 512], bf16, tag="o8")
                    nc.scalar.activation(
                        o8[:], fcps[:], AF.Identity, bias=biases[0:96, 1:2]
                    )
                    ostg = wp.tile([8, 512], f32, tag="ostg")
                    nc.scalar.activation(
                        ostg[:], fcps[0:8, :], AF.Identity,
                        bias=biases[0:8, 2:3],
                    )
                    nc.sync.dma_start(out_d[:, s - T, ms], ostg[:])
                    return o8

                def emit_diff(mc, pt):
                    # graph diffusion: fp8 DoubleRow mask matmul
                    ms = slice(mc * 512, (mc + 1) * 512)
                    pcol = slice(pt * 128, (pt + 1) * 128)
                    dps = dpool.tile([128, 512], f32, tag="dps")
                    for k in range(KT // 2):
                        nc.tensor.matmul(
                            dps[:],
                            hTc[:, 2 * k:2 * k + 2, pcol],
                            mask_sb[:, 2 * k:2 * k + 2, ms],
                            start=(k == 0), stop=(k == KT // 2 - 1),
                            perf_mode=DR,
                        )
                    return dps

                def emit_rest_a(mc, pt, dps, o8):
                    ms = slice(mc * 512, (mc + 1) * 512)
                    # ht = mask_mm * s  (diag rides the exact fp8 pair)
                    ht_sb = wp.tile([128, 512], bf16, tag="htsb")
                    nc.vector.tensor_mul(ht_sb[:], dps[:], sB[:, ms])

                    rps = rzpool.tile([128, 512], f32, tag="rps")
                    zps = nupool.tile([128, 512], f32, tag="zps")
                    nups = nupool.tile([128, 512], f32, tag="nups")
                    nwps = nupool.tile([128, 512], f32, tag="nwps")
                    for g, gps in ((0, rps), (1, zps), (2, nups)):
                        nc.tensor.matmul(
                            gps[:], ublk[:, g, :], ht_sb[:],
                            start=True, stop=(g == 2),
                        )
                    # input terms: row-tiled tiny matmuls (concurrent)
                    for g, gps in ((0, rps), (1, zps), (2, nwps)):
                        if dec:
                            nc.tensor.matmul(
                                gps[:],
                                wdec[32 * g:32 * g + 9, pt * 3 + g, :],
                                o8[32 * g:32 * g + 9, :],
                                start=(g == 2), stop=True,
                            )
                        else:
                            nc.tensor.matmul(
                                gps[:],
                                wx3[32 * g:32 * g + 17, pt * 3 + g, :],
                                x_sb[32 * g:32 * g + 17, ms],
                                start=(g == 2), stop=True,
                            )

                    r = wp.tile([128, 512], bf16, tag="r")
                    nc.scalar.activation(r[:], rps[:], AF.Sigmoid)
                    z = wp.tile([128, 512], bf16, tag="z")
                    nc.scalar.activation(z[:], zps[:], AF.Sigmoid)
                    t1 = wp.tile([128, 512], f32, tag="t1")
                    nc.vector.scalar_tensor_tensor(
                        t1[:], nups[:], biases[:, 0:1], r[:],
                        op0=ALU.add, op1=ALU.mult,
                    )
                    t2 = wp.tile([128, 512], f32, tag="t2")
                    nc.vector.tensor_add(t2[:], nwps[:], t1[:])
                    nw = wp.tile([128, 512], bf16, tag="nw")
                    nc.scalar.activation(nw[:], t2[:], AF.Tanh)
                    # combine needs d = ht - nw early: GpSimd has slack and
                    # keeps it off the DVE critical stream
                    d = wp.tile([128, 512], bf16, tag="d")
                    nc.gpsimd.tensor_sub(d[:], ht_sb[:], nw[:])
                    e = wp.tile([128, 512], bf16, tag="e")
                    nc.gpsimd.tensor_mul(e[:], z[:], d[:])
                    return (mc, pt, e, nw)

                def emit_rest_b(ctx):
                    mc, pt, e, nw = ctx
                    ms = slice(mc * 512, (mc + 1) * 512)
                    pcol = slice(pt * 128, (pt + 1) * 128)
                    # h' = nw + z*(ht - nw)
                    nc.vector.tensor_add(hSn[:, pt, ms], e[:], nw[:])
                    hTb = tb.tile([128, 4, 128], bf16, tag="hTb")
                    nc.sync.dma_start_transpose(hTb[:], hSn[:, pt, ms])
                    nc.scalar.activation(
                        hTn[:, mc * 4:(mc + 1) * 4, pcol], hTb[:], AF.Copy
                    )

                if s == T + P - 1:
                    for mc in range(NMC):
                        emit_fc(mc)   # last step's GRU update is never read
                    continue
                # 3-stage software pipeline: per iteration emit diffusion(i),
                # then state-update tail of chunk i-2, then gates of chunk
                # i-1 — so the static DVE/TensorE streams keep the
                # GpSimd-dependent hSn write behind the next chunk's
                # critical ht_sb/gate ops
                # decoder: all fc projections depend only on the previous
                # step's state, so hoist them to the step start (own o8 pool,
                # bufs=4) — gates never wait on the fc->o8 chain mid-step
                o8s = [emit_fc(mc) for mc in range(NMC)] if dec else None
                pa = pb = None
                for mc in range(NMC):
                    for pt in range(NPT):
                        dps = emit_diff(mc, pt)
                        if pb is not None:
                            emit_rest_b(pb)
                            pb = None
                        if pa is not None:
                            pb = emit_rest_a(*pa)
                        pa = (mc, pt, dps, o8s[mc] if dec else None)
                if pb is not None:
                    emit_rest_b(pb)
                if pa is not None:
                    emit_rest_b(emit_rest_a(*pa))

    nc.compile()
    return nc


def _host_consts(adj, Uw, Ww, Ub, Wb, fc_w, fc_bv):
    """Shared (batch-independent) device inputs, numpy."""
    # adj = diag(d) + s_m * mask, mask in {0,1} exact
    d = np.diag(adj).copy()
    R = adj - np.diag(d)
    s = R.max(axis=0)
    zero = s <= 0
    s[zero] = 1.0
    if DIAG_IN_MASK:
        # adj col m = s_m*(mask + diag(d/s)). Straight fp8 of d/s loses ~2%;
        # instead pick fp8-exact pairs (u, v) with v/u == d/s (e.g. d/s=10/3:
        # u=1.125, v=3.75), store mask*u + diag(v), and fold 1/u into the
        # existing column correction -> fp8 matrix is exact.
        rho = d / s
        ugrid = np.array(
            [1.0 + 0.125 * i for i in range(8)]
            + [0.5 + 0.0625 * i for i in range(8)], np.float32
        )
        cand_v = (rho[None, :] * ugrid[:, None]).astype(_F8).astype(np.float32)
        errs = np.abs(cand_v / (rho[None, :] * ugrid[:, None]) - 1.0)
        j = np.argmin(errs, axis=0)
        u = ugrid[j]
        v = cand_v[j, np.arange(N)]
        maskX = R / s[None, :] * u[None, :] + np.diag(v)
        sB = np.broadcast_to((s / u).astype(np.float32), (128, N)).copy()
    else:
        maskX = R / s[None, :]
        sB = np.broadcast_to(s.astype(np.float32), (128, N)).copy()
    mask8 = np.ascontiguousarray(
        maskX.reshape(KT, 128, N).transpose(1, 0, 2)
    ).astype(_F8)
    dsB = np.broadcast_to((d / s).astype(np.float32), (128, N)).copy()

    # block-diagonal U lhsT, two batches per 128-partition tile
    ublk = np.zeros((128, 3, 128), np.float32)
    for g in range(3):
        for bl in range(2):
            sl = slice(bl * H, (bl + 1) * H)
            ublk[sl, g, sl] = Uw[g].T

    # encoder x-projection lhsT: strip g rows = (b,f) pairs + bias row
    wx3 = np.zeros((96, NPT * 3, 128), np.float32)
    wdec = np.zeros((96, NPT * 3, 128), np.float32)
    for g in range(3):
        bias_g = Ub[g] + Wb[g] if g < 2 else Wb[g]
        for pt in range(NPT):
            col = pt * 3 + g
            for bl in range(2):
                b = pt * 2 + bl
                osl = slice(bl * H, (bl + 1) * H)
                for f in range(F):
                    wx3[32 * g + b * F + f, col, osl] = Ww[g][:, f]
                wdec[32 * g + b, col, osl] = Ww[g][:, 0]
            wx3[32 * g + 16, col, 0:H] = bias_g
            wx3[32 * g + 16, col, H:128] = bias_g
            wdec[32 * g + 8, col, 0:H] = bias_g
            wdec[32 * g + 8, col, H:128] = bias_g

    # decoder fc lhsT: M=96, strip g col 32g+b = batch b
    fcblk = np.zeros((128, NPT, 96), np.float32)
    for g in range(3):
        for pt in range(NPT):
            for bl in range(2):
                b = pt * 2 + bl
                fcblk[bl * H:(bl + 1) * H, pt, 32 * g + b] = fc_w

    biases = np.zeros((128, 4), np.float32)
    for bl in range(2):
        sl = slice(bl * H, (bl + 1) * H)
        biases[sl, 0] = Ub[2]                   # Un_b (inside r-multiply)
    for g in range(3):
        biases[32 * g:32 * g + 8, 1] = fc_bv    # o8 batch rows
        biases[32 * g + 8, 1] = 1.0             # o8 ones-row
    biases[0:8, 2] = fc_bv                      # ostg (f32 output path)

    return dict(
        mask8=mask8, ublk=ublk.astype(_BF16), wx3=wx3.astype(_BF16),
        wdec=wdec.astype(_BF16), fcblk=fcblk.astype(_BF16),
        dsB=dsB, sB=sB, biases=biases,
    )


def _prep_core_inputs(x, hidden0, consts):
    """Per-core input dict for one batch shard (numpy)."""
    # x shard [BC, T, F*N] -> xaug [T, 96, N]: 3 strips of (b,f) + ones row
    xr = x.reshape(BC, T, F, N).transpose(1, 0, 2, 3)   # [T, b, f, N]
    xaug = np.zeros((T, 96, N), np.float32)
    for g in range(3):
        xaug[:, 32 * g:32 * g + 16, :] = xr.reshape(T, 16, N)
        xaug[:, 32 * g + 16, :] = 1.0
    # hidden0 shard [BC, H, N] -> h0s [128=(bl,h), NPT, N]
    h0s = np.ascontiguousarray(
        hidden0.reshape(NPT, 2, H, N).transpose(1, 2, 0, 3).reshape(128, NPT, N)
    )
    # h0t [p, c, pt*128 + i] = h0s[i, pt, 128c + p]
    h0t = np.ascontiguousarray(
        h0s.reshape(128, NPT, KT, 128)      # [i, pt, c, p]
        .transpose(3, 2, 1, 0)              # [p, c, pt, i]
        .reshape(128, KT, 512)
    )
    return dict(
        xaug=xaug.astype(_BF16), h0s=h0s.astype(_BF16), h0t=h0t.astype(_F8),
        **consts,
    )


def kernel(x, hidden0, adj, Ur_w, Ur_b, Wr_w, Wr_b, Uz_w, Uz_b, Wz_w, Wz_b,
           Un_w, Un_b, Wn_w, Wn_b, fc_w, fc_b, horizon):
    global _compiled
    from concourse.bass_utils import run_bass_kernel_spmd

    assert int(horizon) == P
    x = np.asarray(x, np.float32)
    hidden0 = np.asarray(hidden0, np.float32)
    adj = np.asarray(adj, np.float32)

    Uw = [np.asarray(w, np.float32) for w in (Ur_w, Uz_w, Un_w)]
    Ww = [np.asarray(w, np.float32) for w in (Wr_w, Wz_w, Wn_w)]
    Ub = [np.asarray(b, np.float32) for b in (Ur_b, Uz_b, Un_b)]
    Wb = [np.asarray(b, np.float32) for b in (Wr_b, Wz_b, Wn_b)]
    fc_w = np.asarray(fc_w, np.float32).reshape(H)
    fc_bv = float(np.asarray(fc_b, np.float32).reshape(()))

    consts = _host_consts(adj, Uw, Ww, Ub, Wb, fc_w, fc_bv)

    if _compiled is None:
        _compiled = _build_bass()
    nc = _compiled

    in_maps = [
        _prep_core_inputs(
            x[c * BC:(c + 1) * BC], hidden0[c * BC:(c + 1) * BC], consts
        )
        for c in range(NCORES)
    ]
    res = run_bass_kernel_spmd(nc, in_maps, core_ids=list(range(NCORES)))
    out = np.concatenate([res.results[c]["out"] for c in range(NCORES)], axis=0)
    return out.astype(np.float32)


# revision 39
# speedup vs baseline: 1.0089x; 1.0089x over previous
"""Trainium2 Bass kernel for a graph-GRU (GRNN) forecast model.

Math (per batch b, node m, hidden h; N=2048, H=64, F=2, T=12, P=6):
  ht[b,m,:] = sum_n adj[n,m] * h[b,:,n]           (graph diffusion + transpose)
  r = sig(ht@Ur^T + inp@Wr^T + br); z = sig(...); nw = tanh(r*(ht@Un^T+bn1) + inp@Wn^T + bn2)
  h' = (1-z)*nw + z*ht
Encoder: inp = x_t (T steps). Decoder: out = fc(h); inp = [out, 0] (P steps).

Strategy: data-parallel over batch, 8 cores x 8 batches. Per core the state
lives in SBUF in two layouts:
  hT8 [n=2048(part,16 tiles), (b,h)=512(free)] fp8  - lhsT for the diffusion matmul
  hS  [(bl,h)=128(part), pt=4, m=2048]         bf16 - standard layout
with b = 2*pt + bl (batch-pair pt on separate partition tiles).

Key tricks vs a plain bf16 implementation:
 - adj col m = s_m*(mask + diag(d_m/s_m)) with mask in {0,1}: choosing fp8-exact
   pairs (u,v) with v/u == d_m/s_m (e.g. 10/3 -> u=1.125, v=3.75) makes the
   whole fp8 matrix EXACT, so the diffusion runs as fp8 DoubleRow matmuls
   (K=256/inst); the per-column correction s_m/u_m rides the one mandatory
   PSUM->SBUF multiply on DVE.
 - the per-step state transpose uses the DMA xbar (dma_start_transpose) plus a
   ScalarE Copy-cast to fp8, freeing TensorE (~300us of transpose matmuls).
 - gate biases ride ones-rows inside the input-projection matmuls; the three
   tiny input projections (K=17 enc / K=9 dec) are row-tiled onto disjoint
   32-row PE strips so they run concurrently (~1 matmul time instead of 3).
 - decoder input term uses the fc output (computed once per m-chunk, M=96 with
   3 replicated strips) as a K=9 rank-1 matmul instead of full K=128 matmuls.
"""

import numpy as np
import ml_dtypes

B, T, F, N, H, P = 64, 12, 2, 2048, 64, 6
NCORES = 8
BC = B // NCORES          # batches per core = 8
NPT = BC // 2             # batch-pair tiles = 4
KT = N // 128             # contraction tiles = 16
NMC = N // 512            # m chunks = 4

# If True, fold diag(adj)/s into the fp8 mask (one fp8 rounding per column,
# ~2% on the 17% diagonal mass); saves two elementwise ops per chunk.
DIAG_IN_MASK = True

_BF16 = ml_dtypes.bfloat16
_F8 = ml_dtypes.float8_e4m3

_compiled = None


def _build_bass():
    import concourse.bass as bass
    import concourse.mybir as mybir
    from concourse import bacc
    import concourse.tile as tile

    bf16 = mybir.dt.bfloat16
    f32 = mybir.dt.float32
    fp8 = mybir.dt.float8e4
    AF = mybir.ActivationFunctionType
    ALU = mybir.AluOpType
    DR = mybir.MatmulPerfMode.DoubleRow

    nc = bacc.Bacc(None, target_bir_lowering=False)

    x_d = nc.dram_tensor("xaug", [T, 96, N], bf16, kind="ExternalInput")
    h0s_d = nc.dram_tensor("h0s", [128, NPT, N], bf16, kind="ExternalInput")
    h0t_d = nc.dram_tensor("h0t", [128, KT, 512], fp8, kind="ExternalInput")
    mask_d = nc.dram_tensor("mask8", [128, KT, N], fp8, kind="ExternalInput")
    ublk_d = nc.dram_tensor("ublk", [128, 3, 128], bf16, kind="ExternalInput")
    wx3_d = nc.dram_tensor("wx3", [96, NPT * 3, 128], bf16, kind="ExternalInput")
    wdec_d = nc.dram_tensor("wdec", [96, NPT * 3, 128], bf16, kind="ExternalInput")
    fcb_d = nc.dram_tensor("fcblk", [128, NPT, 96], bf16, kind="ExternalInput")
    ds_d = nc.dram_tensor("dsB", [128, N], f32, kind="ExternalInput")
    s_d = nc.dram_tensor("sB", [128, N], f32, kind="ExternalInput")
    bias_d = nc.dram_tensor("biases", [128, 4], f32, kind="ExternalInput")
    out_d = nc.dram_tensor("out", [BC, P, N], f32, kind="ExternalOutput")

    with tile.TileContext(nc) as tc:
        with (
            tc.tile_pool(name="const", bufs=1) as cp,
            tc.tile_pool(name="state", bufs=1) as sp,
            tc.tile_pool(name="work", bufs=3) as wp,
            tc.tile_pool(name="xp", bufs=3) as xp,
            tc.tile_pool(name="tb", bufs=4) as tb,
            tc.tile_pool(name="dps", bufs=3, space="PSUM") as dpool,
            tc.tile_pool(name="rz", bufs=2, space="PSUM") as rzpool,
            tc.tile_pool(name="nunw", bufs=1, space="PSUM") as nupool,
        ):
            # load order matters: the step-0 diffusion needs h0t + mask first
            hT = [sp.tile([128, KT, 512], fp8, name=f"hT{i}") for i in range(2)]
            hS = [sp.tile([128, NPT, N], bf16, name=f"hS{i}") for i in range(2)]
            nc.sync.dma_start(hT[0][:], h0t_d[:])
            mask_sb = cp.tile([128, KT, N], fp8)
            for kt in range(KT):
                eng = (nc.sync, nc.scalar, nc.gpsimd)[kt % 3]
                eng.dma_start(mask_sb[:, kt, :], mask_d[:, kt, :])
            ublk = cp.tile([128, 3, 128], bf16)
            nc.sync.dma_start(ublk[:], ublk_d[:])
            wx3 = cp.tile([96, NPT * 3, 128], bf16)
            nc.sync.dma_start(wx3[:], wx3_d[:])
            sB = cp.tile([128, N], f32)
            nc.sync.dma_start(sB[:], s_d[:])
            biases = cp.tile([128, 4], f32)
            nc.sync.dma_start(biases[:], bias_d[:])
            wdec = cp.tile([96, NPT * 3, 128], bf16)
            nc.sync.dma_start(wdec[:], wdec_d[:])
            fcblk = cp.tile([128, NPT, 96], bf16)
            nc.sync.dma_start(fcblk[:], fcb_d[:])
            dsB = cp.tile([128, N], f32)
            nc.sync.dma_start(dsB[:], ds_d[:])
            nc.sync.dma_start(hS[0][:], h0s_d[:])

            for s in range(T + P):
                cur, nxt = s % 2, (s + 1) % 2
                dec = s >= T
                hTc, hTn = hT[cur], hT[nxt]
                hSp, hSn = hS[cur], hS[nxt]
                if not dec:
                    x_sb = xp.tile([96, N], bf16, tag="xsb")
                    nc.sync.dma_start(x_sb[:], x_d[s, :, :])
                else:
                    p_idx = s - T
                def emit_fc(mc):
                    # fc out-projection: M=96 (3 replicated 8-batch strips
                    # + ones-rows via bias), accumulated over pt.
                    # Borrows the rps bank (cleared by the next start=True).
                    ms = slice(mc * 512, (mc + 1) * 512)
                    fct = rzpool.tile([128, 512], f32, tag="rps", name="fct")
                    fcps = fct[0:96, :]
                    for pt in range(NPT):
                        nc.tensor.matmul(
                            fcps[:], fcblk[:, pt, :], hSp[:, pt, ms],
                            start=(pt == 0), stop=(pt == NPT - 1),
                        )
                    o8 = wp.tile([96, 512], bf16, tag="o8")
                    nc.scalar.activation(
                        o8[:], fcps[:], AF.Identity, bias=biases[0:96, 1:2]
                    )
                    ostg = wp.tile([8, 512], f32, tag="ostg")
                    nc.scalar.activation(
                        ostg[:], fcps[0:8, :], AF.Identity,
                        bias=biases[0:8, 2:3],
                    )
                    nc.sync.dma_start(out_d[:, s - T, ms], ostg[:])
                    return o8

                def emit_diff(mc, pt):
                    # graph diffusion: fp8 DoubleRow mask matmul
                    ms = slice(mc * 512, (mc + 1) * 512)
                    pcol = slice(pt * 128, (pt + 1) * 128)
                    dps = dpool.tile([128, 512], f32, tag="dps")
                    for k in range(KT // 2):
                        nc.tensor.matmul(
                            dps[:],
                            hTc[:, 2 * k:2 * k + 2, pcol],
                            mask_sb[:, 2 * k:2 * k + 2, ms],
                            start=(k == 0), stop=(k == KT // 2 - 1),
                            perf_mode=DR,
                        )
                    return dps

                def emit_rest_a(mc, pt, dps, o8):
                    ms = slice(mc * 512, (mc + 1) * 512)
                    # ht = mask_mm * s  (diag rides the exact fp8 pair)
                    ht_sb = wp.tile([128, 512], bf16, tag="htsb")
                    nc.vector.tensor_mul(ht_sb[:], dps[:], sB[:, ms])

                    rps = rzpool.tile([128, 512], f32, tag="rps")
                    zps = nupool.tile([128, 512], f32, tag="zps")
                    nups = nupool.tile([128, 512], f32, tag="nups")
                    nwps = nupool.tile([128, 512], f32, tag="nwps")
                    for g, gps in ((0, rps), (1, zps), (2, nups)):
                        nc.tensor.matmul(
                            gps[:], ublk[:, g, :], ht_sb[:],
                            start=True, stop=(g == 2),
                        )
                    # input terms: row-tiled tiny matmuls (concurrent)
                    for g, gps in ((0, rps), (1, zps), (2, nwps)):
                        if dec:
                            nc.tensor.matmul(
                                gps[:],
                                wdec[32 * g:32 * g + 9, pt * 3 + g, :],
                                o8[32 * g:32 * g + 9, :],
                                start=(g == 2), stop=True,
                            )
                        else:
                            nc.tensor.matmul(
                                gps[:],
                                wx3[32 * g:32 * g + 17, pt * 3 + g, :],
                                x_sb[32 * g:32 * g + 17, ms],
                                start=(g == 2), stop=True,
                            )

                    r = wp.tile([128, 512], bf16, tag="r")
                    nc.scalar.activation(r[:], rps[:], AF.Sigmoid)
                    z = wp.tile([128, 512], bf16, tag="z")
                    nc.scalar.activation(z[:], zps[:], AF.Sigmoid)
                    t1 = wp.tile([128, 512], f32, tag="t1")
                    nc.vector.scalar_tensor_tensor(
                        t1[:], nups[:], biases[:, 0:1], r[:],
                        op0=ALU.add, op1=ALU.mult,
                    )
                    t2 = wp.tile([128, 512], f32, tag="t2")
                    nc.vector.tensor_add(t2[:], nwps[:], t1[:])
                    nw = wp.tile([128, 512], bf16, tag="nw")
                    nc.scalar.activation(nw[:], t2[:], AF.Tanh)
                    # combine needs d = ht - nw early: GpSimd has slack and
                    # keeps it off the DVE critical stream
                    d = wp.tile([128, 512], bf16, tag="d")
                    nc.gpsimd.tensor_sub(d[:], ht_sb[:], nw[:])
                    e = wp.tile([128, 512], bf16, tag="e")
                    nc.gpsimd.tensor_mul(e[:], z[:], d[:])
                    return (mc, pt, e, nw)

                def emit_rest_b(ctx):
                    mc, pt, e, nw = ctx
                    ms = slice(mc * 512, (mc + 1) * 512)
                    pcol = slice(pt * 128, (pt + 1) * 128)
                    # h' = nw + z*(ht - nw)
                    nc.vector.tensor_add(hSn[:, pt, ms], e[:], nw[:])
                    hTb = tb.tile([128, 4, 128], bf16, tag="hTb")
                    nc.sync.dma_start_transpose(hTb[:], hSn[:, pt, ms])
                    nc.scalar.activation(
                        hTn[:, mc * 4:(mc + 1) * 4, pcol], hTb[:], AF.Copy
                    )

                if s == T + P - 1:
                    for mc in range(NMC):
                        emit_fc(mc)   # last step's GRU update is never read
                    continue
                # 3-stage software pipeline: per iteration emit diffusion(i),
                # then state-update tail of chunk i-2, then gates of chunk
                # i-1 — so the static DVE/TensorE streams keep the
                # GpSimd-dependent hSn write behind the next chunk's
                # critical ht_sb/gate ops
                pa = pb = None
                o8cur = None
                for mc in range(NMC):
                    if dec:
                        o8cur = emit_fc(mc)
                    for pt in range(NPT):
                        dps = emit_diff(mc, pt)
                        if pb is not None:
                            emit_rest_b(pb)
                            pb = None
                        if pa is not None:
                            pb = emit_rest_a(*pa)
                        pa = (mc, pt, dps, o8cur)
                if pb is not None:
                    emit_rest_b(pb)
                if pa is not None:
                    emit_rest_b(emit_rest_a(*pa))

    nc.compile()
    return nc


def _host_consts(adj, Uw, Ww, Ub, Wb, fc_w, fc_bv):
    """Shared (batch-independent) device inputs, numpy."""
    # adj = diag(d) + s_m * mask, mask in {0,1} exact
    d = np.diag(adj).copy()
    R = adj - np.diag(d)
    s = R.max(axis=0)
    zero = s <= 0
    s[zero] = 1.0
    if DIAG_IN_MASK:
        # adj col m = s_m*(mask + diag(d/s)). Straight fp8 of d/s loses ~2%;
        # instead pick fp8-exact pairs (u, v) with v/u == d/s (e.g. d/s=10/3:
        # u=1.125, v=3.75), store mask*u + diag(v), and fold 1/u into the
        # existing column correction -> fp8 matrix is exact.
        rho = d / s
        ugrid = np.array(
            [1.0 + 0.125 * i for i in range(8)]
            + [0.5 + 0.0625 * i for i in range(8)], np.float32
        )
        cand_v = (rho[None, :] * ugrid[:, None]).astype(_F8).astype(np.float32)
        errs = np.abs(cand_v / (rho[None, :] * ugrid[:, None]) - 1.0)
        j = np.argmin(errs, axis=0)
        u = ugrid[j]
        v = cand_v[j, np.arange(N)]
        maskX = R / s[None, :] * u[None, :] + np.diag(v)
        sB = np.broadcast_to((s / u).astype(np.float32), (128, N)).copy()
    else:
        maskX = R / s[None, :]
        sB = np.broadcast_to(s.astype(np.float32), (128, N)).copy()
    mask8 = np.ascontiguousarray(
        maskX.reshape(KT, 128, N).transpose(1, 0, 2)
    ).astype(_F8)
    dsB = np.broadcast_to((d / s).astype(np.float32), (128, N)).copy()

    # block-diagonal U lhsT, two batches per 128-partition tile
    ublk = np.zeros((128, 3, 128), np.float32)
    for g in range(3):
        for bl in range(2):
            sl = slice(bl * H, (bl + 1) * H)
            ublk[sl, g, sl] = Uw[g].T

    # encoder x-projection lhsT: strip g rows = (b,f) pairs + bias row
    wx3 = np.zeros((96, NPT * 3, 128), np.float32)
    wdec = np.zeros((96, NPT * 3, 128), np.float32)
    for g in range(3):
        bias_g = Ub[g] + Wb[g] if g < 2 else Wb[g]
        for pt in range(NPT):
            col = pt * 3 + g
            for bl in range(2):
                b = pt * 2 + bl
                osl = slice(bl * H, (bl + 1) * H)
                for f in range(F):
                    wx3[32 * g + b * F + f, col, osl] = Ww[g][:, f]
                wdec[32 * g + b, col, osl] = Ww[g][:, 0]
            wx3[32 * g + 16, col, 0:H] = bias_g
            wx3[32 * g + 16, col, H:128] = bias_g
            wdec[32 * g + 8, col, 0:H] = bias_g
            wdec[32 * g + 8, col, H:128] = bias_g

    # decoder fc lhsT: M=96, strip g col 32g+b = batch b
    fcblk = np.zeros((128, NPT, 96), np.float32)
    for g in range(3):
        for pt in range(NPT):
            for bl in range(2):
                b = pt * 2 + bl
                fcblk[bl * H:(bl + 1) * H, pt, 32 * g + b] = fc_w

    biases = np.zeros((128, 4), np.float32)
    for bl in range(2):
        sl = slice(bl * H, (bl + 1) * H)
        biases[sl, 0] = Ub[2]                   # Un_b (inside r-multiply)
    for g in range(3):
        biases[32 * g:32 * g + 8, 1] = fc_bv    # o8 batch rows
        biases[32 * g + 8, 1] = 1.0             # o8 ones-row
    biases[0:8, 2] = fc_bv                      # ostg (f32 output path)

    return dict(
        mask8=mask8, ublk=ublk.astype(_BF16), wx3=wx3.astype(_BF16),
        wdec=wdec.astype(_BF16), fcblk=fcblk.astype(_BF16),
        dsB=dsB, sB=sB, biases=biases,
    )


def _prep_core_inputs(x, hidden0, consts):
    """Per-core input dict for one batch shard (numpy)."""
    # x shard [BC, T, F*N] -> xaug [T, 96, N]: 3 strips of (b,f) + ones row
    xr = x.reshape(BC, T, F, N).transpose(1, 0, 2, 3)   # [T, b, f, N]
    xaug = np.zeros((T, 96, N), np.float32)
    for g in range(3):
        xaug[:, 32 * g:32 * g + 16, :] = xr.reshape(T, 16, N)
        xaug[:, 32 * g + 16, :] = 1.0
    # hidden0 shard [BC, H, N] -> h0s [128=(bl,h), NPT, N]
    h0s = np.ascontiguousarray(
        hidden0.reshape(NPT, 2, H, N).transpose(1, 2, 0, 3).reshape(128, NPT, N)
    )
    # h0t [p, c, pt*128 + i] = h0s[i, pt, 128c + p]
    h0t = np.ascontiguousarray(
        h0s.reshape(128, NPT, KT, 128)      # [i, pt, c, p]
        .transpose(3, 2, 1, 0)              # [p, c, pt, i]
        .reshape(128, KT, 512)
    )
    return dict(
        xaug=xaug.astype(_BF16), h0s=h0s.astype(_BF16), h0t=h0t.astype(_F8),
        **consts,
    )


def kernel(x, hidden0, adj, Ur_w, Ur_b, Wr_w, Wr_b, Uz_w, Uz_b, Wz_w, Wz_b,
           Un_w, Un_b, Wn_w, Wn_b, fc_w, fc_b, horizon):
    global _compiled
    from concourse.bass_utils import run_bass_kernel_spmd

    assert int(horizon) == P
    x = np.asarray(x, np.float32)
    hidden0 = np.asarray(hidden0, np.float32)
    adj = np.asarray(adj, np.float32)

    Uw = [np.asarray(w, np.float32) for w in (Ur_w, Uz_w, Un_w)]
    Ww = [np.asarray(w, np.float32) for w in (Wr_w, Wz_w, Wn_w)]
    Ub = [np.asarray(b, np.float32) for b in (Ur_b, Uz_b, Un_b)]
    Wb = [np.asarray(b, np.float32) for b in (Wr_b, Wz_b, Wn_b)]
    fc_w = np.asarray(fc_w, np.float32).reshape(H)
    fc_bv = float(np.asarray(fc_b, np.float32).reshape(()))

    consts = _host_consts(adj, Uw, Ww, Ub, Wb, fc_w, fc_bv)

    if _compiled is None:
        _compiled = _build_bass()
    nc = _compiled

    in_maps = [
        _prep_core_inputs(
            x[c * BC:(c + 1) * BC], hidden0[c * BC:(c + 1) * BC], consts
        )
        for c in range(NCORES)
    ]
    res = run_bass_kernel_spmd(nc, in_maps, core_ids=list(range(NCORES)))
    out = np.concatenate([res.results[c]["out"] for c in range(NCORES)], axis=0)
    return out.astype(np.float32)


# revision 40
# speedup vs baseline: 1.1759x; 1.1655x over previous
"""Trainium2 Bass kernel for a graph-GRU (GRNN) forecast model.

Math (per batch b, node m, hidden h; N=2048, H=64, F=2, T=12, P=6):
  ht[b,m,:] = sum_n adj[n,m] * h[b,:,n]           (graph diffusion + transpose)
  r = sig(ht@Ur^T + inp@Wr^T + br); z = sig(...); nw = tanh(r*(ht@Un^T+bn1) + inp@Wn^T + bn2)
  h' = (1-z)*nw + z*ht
Encoder: inp = x_t (T steps). Decoder: out = fc(h); inp = [out, 0] (P steps).

Strategy: data-parallel over batch, 8 cores x 8 batches. Per core the state
lives in SBUF in two layouts:
  hT8 [n=2048(part,16 tiles), (b,h)=512(free)] fp8  - lhsT for the diffusion matmul
  hS  [(bl,h)=128(part), pt=4, m=2048]         bf16 - standard layout
with b = 2*pt + bl (batch-pair pt on separate partition tiles).

Key tricks vs a plain bf16 implementation:
 - adj col m = s_m*(mask + diag(d_m/s_m)) with mask in {0,1}: choosing fp8-exact
   pairs (u,v) with v/u == d_m/s_m (e.g. 10/3 -> u=1.125, v=3.75) makes the
   whole fp8 matrix EXACT, so the diffusion runs as fp8 DoubleRow matmuls
   (K=256/inst); the per-column correction s_m/u_m rides the one mandatory
   PSUM->SBUF multiply on DVE.
 - the per-step state transpose uses the DMA xbar (dma_start_transpose) plus a
   ScalarE Copy-cast to fp8, freeing TensorE (~300us of transpose matmuls).
 - gate biases ride ones-rows inside the input-projection matmuls; the three
   tiny input projections (K=17 enc / K=9 dec) are row-tiled onto disjoint
   32-row PE strips so they run concurrently (~1 matmul time instead of 3).
 - decoder input term uses the fc output (computed once per m-chunk, M=96 with
   3 replicated strips) as a K=9 rank-1 matmul instead of full K=128 matmuls.
"""

import numpy as np
import ml_dtypes

B, T, F, N, H, P = 64, 12, 2, 2048, 64, 6
NCORES = 8
BC = B // NCORES          # batches per core = 8
NPT = BC // 2             # batch-pair tiles = 4
KT = N // 128             # contraction tiles = 16
NMC = N // 512            # m chunks = 4

# If True, fold diag(adj)/s into the fp8 mask (one fp8 rounding per column,
# ~2% on the 17% diagonal mass); saves two elementwise ops per chunk.
DIAG_IN_MASK = True

_BF16 = ml_dtypes.bfloat16
_F8 = ml_dtypes.float8_e4m3

_compiled = None


def _build_bass():
    import concourse.bass as bass
    import concourse.mybir as mybir
    from concourse import bacc
    import concourse.tile as tile

    bf16 = mybir.dt.bfloat16
    f32 = mybir.dt.float32
    fp8 = mybir.dt.float8e4
    AF = mybir.ActivationFunctionType
    ALU = mybir.AluOpType
    DR = mybir.MatmulPerfMode.DoubleRow

    nc = bacc.Bacc(None, target_bir_lowering=False)

    x_d = nc.dram_tensor("xaug", [T, 96, N], bf16, kind="ExternalInput")
    h0s_d = nc.dram_tensor("h0s", [128, NPT, N], bf16, kind="ExternalInput")
    h0t_d = nc.dram_tensor("h0t", [128, KT, 512], fp8, kind="ExternalInput")
    mask_d = nc.dram_tensor("mask8", [128, KT, N], fp8, kind="ExternalInput")
    ublk_d = nc.dram_tensor("ublk", [128, 3, 128], bf16, kind="ExternalInput")
    wx3_d = nc.dram_tensor("wx3", [96, NPT * 3, 128], bf16, kind="ExternalInput")
    wdec_d = nc.dram_tensor("wdec", [96, NPT * 3, 128], bf16, kind="ExternalInput")
    fcb_d = nc.dram_tensor("fcblk", [128, NPT, 96], bf16, kind="ExternalInput")
    ds_d = nc.dram_tensor("dsB", [128, N], f32, kind="ExternalInput")
    s_d = nc.dram_tensor("sB", [128, N], f32, kind="ExternalInput")
    bias_d = nc.dram_tensor("biases", [128, 4], f32, kind="ExternalInput")
    out_d = nc.dram_tensor("out", [BC, P, N], f32, kind="ExternalOutput")

    with tile.TileContext(nc) as tc:
        with (
            tc.tile_pool(name="const", bufs=1) as cp,
            tc.tile_pool(name="state", bufs=1) as sp,
            tc.tile_pool(name="work", bufs=3) as wp,
            tc.tile_pool(name="xp", bufs=2) as xp,
            tc.tile_pool(name="tb", bufs=3) as tb,
            tc.tile_pool(name="dps", bufs=3, space="PSUM") as dpool,
            tc.tile_pool(name="rz", bufs=2, space="PSUM") as rzpool,
            tc.tile_pool(name="nunw", bufs=1, space="PSUM") as nupool,
        ):
            # load order matters: the step-0 diffusion needs h0t + mask first
            hT = [sp.tile([128, KT, 512], fp8, name=f"hT{i}") for i in range(2)]
            hS = [sp.tile([128, NPT, N], bf16, name=f"hS{i}") for i in range(2)]
            nc.sync.dma_start(hT[0][:], h0t_d[:])
            mask_sb = cp.tile([128, KT, N], fp8)
            for kt in range(KT):
                eng = nc.sync if kt % 2 == 0 else nc.scalar
                eng.dma_start(mask_sb[:, kt, :], mask_d[:, kt, :])
            ublk = cp.tile([128, 3, 128], bf16)
            nc.sync.dma_start(ublk[:], ublk_d[:])
            wx3 = cp.tile([96, NPT * 3, 128], bf16)
            nc.sync.dma_start(wx3[:], wx3_d[:])
            sB = cp.tile([128, N], f32)
            nc.sync.dma_start(sB[:], s_d[:])
            biases = cp.tile([128, 4], f32)
            nc.sync.dma_start(biases[:], bias_d[:])
            wdec = cp.tile([96, NPT * 3, 128], bf16)
            nc.sync.dma_start(wdec[:], wdec_d[:])
            fcblk = cp.tile([128, NPT, 96], bf16)
            nc.sync.dma_start(fcblk[:], fcb_d[:])
            dsB = cp.tile([128, N], f32)
            nc.sync.dma_start(dsB[:], ds_d[:])
            nc.sync.dma_start(hS[0][:], h0s_d[:])

            for s in range(T + P):
                cur, nxt = s % 2, (s + 1) % 2
                dec = s >= T
                hTc, hTn = hT[cur], hT[nxt]
                hSp, hSn = hS[cur], hS[nxt]
                if not dec:
                    x_sb = xp.tile([96, N], bf16, tag="xsb")
                    nc.sync.dma_start(x_sb[:], x_d[s, :, :])
                else:
                    p_idx = s - T
                def emit_fc(mc):
                    # fc out-projection: M=96 (3 replicated 8-batch strips
                    # + ones-rows via bias), accumulated over pt.
                    # Borrows the rps bank (cleared by the next start=True).
                    ms = slice(mc * 512, (mc + 1) * 512)
                    fct = rzpool.tile([128, 512], f32, tag="rps", name="fct")
                    fcps = fct[0:96, :]
                    for pt in range(NPT):
                        nc.tensor.matmul(
                            fcps[:], fcblk[:, pt, :], hSp[:, pt, ms],
                            start=(pt == 0), stop=(pt == NPT - 1),
                        )
                    o8 = wp.tile([96, 512], bf16, tag="o8")
                    nc.scalar.activation(
                        o8[:], fcps[:], AF.Identity, bias=biases[0:96, 1:2]
                    )
                    ostg = wp.tile([8, 512], f32, tag="ostg")
                    nc.scalar.activation(
                        ostg[:], fcps[0:8, :], AF.Identity,
                        bias=biases[0:8, 2:3],
                    )
                    nc.sync.dma_start(out_d[:, s - T, ms], ostg[:])
                    return o8

                def emit_diff(mc, pt):
                    # graph diffusion: fp8 DoubleRow mask matmul
                    ms = slice(mc * 512, (mc + 1) * 512)
                    pcol = slice(pt * 128, (pt + 1) * 128)
                    dps = dpool.tile([128, 512], f32, tag="dps")
                    for k in range(KT // 2):
                        nc.tensor.matmul(
                            dps[:],
                            hTc[:, 2 * k:2 * k + 2, pcol],
                            mask_sb[:, 2 * k:2 * k + 2, ms],
                            start=(k == 0), stop=(k == KT // 2 - 1),
                            perf_mode=DR,
                        )
                    return dps

                def emit_rest_a(mc, pt, dps, o8):
                    ms = slice(mc * 512, (mc + 1) * 512)
                    # ht = mask_mm * s  (diag rides the exact fp8 pair)
                    ht_sb = wp.tile([128, 512], bf16, tag="htsb")
                    nc.vector.tensor_mul(ht_sb[:], dps[:], sB[:, ms])

                    rps = rzpool.tile([128, 512], f32, tag="rps")
                    zps = nupool.tile([128, 512], f32, tag="zps")
                    nups = nupool.tile([128, 512], f32, tag="nups")
                    nwps = nupool.tile([128, 512], f32, tag="nwps")
                    for g, gps in ((0, rps), (1, zps), (2, nups)):
                        nc.tensor.matmul(
                            gps[:], ublk[:, g, :], ht_sb[:],
                            start=True, stop=(g == 2),
                        )
                    # input terms: row-tiled tiny matmuls (concurrent)
                    for g, gps in ((0, rps), (1, zps), (2, nwps)):
                        if dec:
                            nc.tensor.matmul(
                                gps[:],
                                wdec[32 * g:32 * g + 9, pt * 3 + g, :],
                                o8[32 * g:32 * g + 9, :],
                                start=(g == 2), stop=True,
                            )
                        else:
                            nc.tensor.matmul(
                                gps[:],
                                wx3[32 * g:32 * g + 17, pt * 3 + g, :],
                                x_sb[32 * g:32 * g + 17, ms],
                                start=(g == 2), stop=True,
                            )

                    r = wp.tile([128, 512], bf16, tag="r")
                    nc.scalar.activation(r[:], rps[:], AF.Sigmoid)
                    z = wp.tile([128, 512], bf16, tag="z")
                    nc.scalar.activation(z[:], zps[:], AF.Sigmoid)
                    t1 = wp.tile([128, 512], f32, tag="t1")
                    nc.vector.scalar_tensor_tensor(
                        t1[:], nups[:], biases[:, 0:1], r[:],
                        op0=ALU.add, op1=ALU.mult,
                    )
                    t2 = wp.tile([128, 512], f32, tag="t2")
                    nc.vector.tensor_add(t2[:], nwps[:], t1[:])
                    nw = wp.tile([128, 512], bf16, tag="nw")
                    nc.scalar.activation(nw[:], t2[:], AF.Tanh)
                    # combine needs d = ht - nw early: GpSimd has slack and
                    # keeps it off the DVE critical stream
                    d = wp.tile([128, 512], bf16, tag="d")
                    nc.gpsimd.tensor_sub(d[:], ht_sb[:], nw[:])
                    e = wp.tile([128, 512], bf16, tag="e")
                    nc.gpsimd.tensor_mul(e[:], z[:], d[:])
                    return (mc, pt, e, nw)

                def emit_rest_b(ctx):
                    mc, pt, e, nw = ctx
                    ms = slice(mc * 512, (mc + 1) * 512)
                    pcol = slice(pt * 128, (pt + 1) * 128)
                    # h' = nw + z*(ht - nw)
                    nc.vector.tensor_add(hSn[:, pt, ms], e[:], nw[:])
                    hTb = tb.tile([128, 4, 128], bf16, tag="hTb")
                    nc.sync.dma_start_transpose(hTb[:], hSn[:, pt, ms])
                    nc.scalar.activation(
                        hTn[:, mc * 4:(mc + 1) * 4, pcol], hTb[:], AF.Copy
                    )

                if s == T + P - 1:
                    for mc in range(NMC):
                        emit_fc(mc)   # last step's GRU update is never read
                    continue
                # 3-stage software pipeline: per iteration emit diffusion(i),
                # then state-update tail of chunk i-2, then gates of chunk
                # i-1 — so the static DVE/TensorE streams keep the
                # GpSimd-dependent hSn write behind the next chunk's
                # critical ht_sb/gate ops
                pa = pb = None
                o8cur = None
                for mc in range(NMC):
                    if dec:
                        o8cur = emit_fc(mc)
                    for pt in range(NPT):
                        dps = emit_diff(mc, pt)
                        if pb is not None:
                            emit_rest_b(pb)
                            pb = None
                        if pa is not None:
                            pb = emit_rest_a(*pa)
                        pa = (mc, pt, dps, o8cur)
                if pb is not None:
                    emit_rest_b(pb)
                if pa is not None:
                    emit_rest_b(emit_rest_a(*pa))

    nc.compile()
    return nc


def _host_consts(adj, Uw, Ww, Ub, Wb, fc_w, fc_bv):
    """Shared (batch-independent) device inputs, numpy."""
    # adj = diag(d) + s_m * mask, mask in {0,1} exact
    d = np.diag(adj).copy()
    R = adj - np.diag(d)
    s = R.max(axis=0)
    zero = s <= 0
    s[zero] = 1.0
    if DIAG_IN_MASK:
        # adj col m = s_m*(mask + diag(d/s)). Straight fp8 of d/s loses ~2%;
        # instead pick fp8-exact pairs (u, v) with v/u == d/s (e.g. d/s=10/3:
        # u=1.125, v=3.75), store mask*u + diag(v), and fold 1/u into the
        # existing column correction -> fp8 matrix is exact.
        rho = d / s
        ugrid = np.array(
            [1.0 + 0.125 * i for i in range(8)]
            + [0.5 + 0.0625 * i for i in range(8)], np.float32
        )
        cand_v = (rho[None, :] * ugrid[:, None]).astype(_F8).astype(np.float32)
        errs = np.abs(cand_v / (rho[None, :] * ugrid[:, None]) - 1.0)
        j = np.argmin(errs, axis=0)
        u = ugrid[j]
        v = cand_v[j, np.arange(N)]
        maskX = R / s[None, :] * u[None, :] + np.diag(v)
        sB = np.broadcast_to((s / u).astype(np.float32), (128, N)).copy()
    else:
        maskX = R / s[None, :]
        sB = np.broadcast_to(s.astype(np.float32), (128, N)).copy()
    mask8 = np.ascontiguousarray(
        maskX.reshape(KT, 128, N).transpose(1, 0, 2)
    ).astype(_F8)
    dsB = np.broadcast_to((d / s).astype(np.float32), (128, N)).copy()

    # block-diagonal U lhsT, two batches per 128-partition tile
    ublk = np.zeros((128, 3, 128), np.float32)
    for g in range(3):
        for bl in range(2):
            sl = slice(bl * H, (bl + 1) * H)
            ublk[sl, g, sl] = Uw[g].T

    # encoder x-projection lhsT: strip g rows = (b,f) pairs + bias row
    wx3 = np.zeros((96, NPT * 3, 128), np.float32)
    wdec = np.zeros((96, NPT * 3, 128), np.float32)
    for g in range(3):
        bias_g = Ub[g] + Wb[g] if g < 2 else Wb[g]
        for pt in range(NPT):
            col = pt * 3 + g
            for bl in range(2):
                b = pt * 2 + bl
                osl = slice(bl * H, (bl + 1) * H)
                for f in range(F):
                    wx3[32 * g + b * F + f, col, osl] = Ww[g][:, f]
                wdec[32 * g + b, col, osl] = Ww[g][:, 0]
            wx3[32 * g + 16, col, 0:H] = bias_g
            wx3[32 * g + 16, col, H:128] = bias_g
            wdec[32 * g + 8, col, 0:H] = bias_g
            wdec[32 * g + 8, col, H:128] = bias_g

    # decoder fc lhsT: M=96, strip g col 32g+b = batch b
    fcblk = np.zeros((128, NPT, 96), np.float32)
    for g in range(3):
        for pt in range(NPT):
            for bl in range(2):
                b = pt * 2 + bl
                fcblk[bl * H:(bl + 1) * H, pt, 32 * g + b] = fc_w

    biases = np.zeros((128, 4), np.float32)
    for bl in range(2):
        sl = slice(bl * H, (bl + 1) * H)
        biases[sl, 0] = Ub[2]                   # Un_b (inside r-multiply)
    for g in range(3):
        biases[32 * g:32 * g + 8, 1] = fc_bv    # o8 batch rows
        biases[32 * g + 8, 1] = 1.0             # o8 ones-row
    biases[0:8, 2] = fc_bv                      # ostg (f32 output path)

    return dict(
        mask8=mask8, ublk=ublk.astype(_BF16), wx3=wx3.astype(_BF16),
        wdec=wdec.astype(_BF16), fcblk=fcblk.astype(_BF16),
        dsB=dsB, sB=sB, biases=biases,
    )


def _prep_core_inputs(x, hidden0, consts):
    """Per-core input dict for one batch shard (numpy)."""
    # x shard [BC, T, F*N] -> xaug [T, 96, N]: 3 strips of (b,f) + ones row
    xr = x.reshape(BC, T, F, N).transpose(1, 0, 2, 3)   # [T, b, f, N]
    xaug = np.zeros((T, 96, N), np.float32)
    for g in range(3):
        xaug[:, 32 * g:32 * g + 16, :] = xr.reshape(T, 16, N)
        xaug[:, 32 * g + 16, :] = 1.0
    # hidden0 shard [BC, H, N] -> h0s [128=(bl,h), NPT, N]
    h0s = np.ascontiguousarray(
        hidden0.reshape(NPT, 2, H, N).transpose(1, 2, 0, 3).reshape(128, NPT, N)
    )
    # h0t [p, c, pt*128 + i] = h0s[i, pt, 128c + p]
    h0t = np.ascontiguousarray(
        h0s.reshape(128, NPT, KT, 128)      # [i, pt, c, p]
        .transpose(3, 2, 1, 0)              # [p, c, pt, i]
        .reshape(128, KT, 512)
    )
    return dict(
        xaug=xaug.astype(_BF16), h0s=h0s.astype(_BF16), h0t=h0t.astype(_F8),
        **consts,
    )


def kernel(x, hidden0, adj, Ur_w, Ur_b, Wr_w, Wr_b, Uz_w, Uz_b, Wz_w, Wz_b,
           Un_w, Un_b, Wn_w, Wn_b, fc_w, fc_b, horizon):
    global _compiled
    from concourse.bass_utils import run_bass_kernel_spmd

    assert int(horizon) == P
    x = np.asarray(x, np.float32)
    hidden0 = np.asarray(hidden0, np.float32)
    adj = np.asarray(adj, np.float32)

    Uw = [np.asarray(w, np.float32) for w in (Ur_w, Uz_w, Un_w)]
    Ww = [np.asarray(w, np.float32) for w in (Wr_w, Wz_w, Wn_w)]
    Ub = [np.asarray(b, np.float32) for b in (Ur_b, Uz_b, Un_b)]
    Wb = [np.asarray(b, np.float32) for b in (Wr_b, Wz_b, Wn_b)]
    fc_w = np.asarray(fc_w, np.float32).reshape(H)
    fc_bv = float(np.asarray(fc_b, np.float32).reshape(()))

    consts = _host_consts(adj, Uw, Ww, Ub, Wb, fc_w, fc_bv)

    if _compiled is None:
        _compiled = _build_bass()
    nc = _compiled

    in_maps = [
        _prep_core_inputs(
            x[c * BC:(c + 1) * BC], hidden0[c * BC:(c + 1) * BC], consts
        )
        for c in range(NCORES)
    ]
    res = run_bass_kernel_spmd(nc, in_maps, core_ids=list(range(NCORES)))
    out = np.concatenate([res.results[c]["out"] for c in range(NCORES)], axis=0)
    return out.astype(np.float32)
